# revision 34
# baseline (speedup 1.0000x reference)
"""Multi-head causal self-attention (B=2, T=4096, D=768, H=12) on 8 trn2 cores.

Sharding: core c -> batch b = c//4, heads 3*(c%4) .. 3*(c%4)+2.
qkv_proj column-parallel (each core computes Q/K/V only for its heads),
out_proj row-parallel (each core emits a partial y^T; host sums the 4
partials per batch).

v6: the ScalarE exp stream (~1us per [128,1024] score tile) is the
bottleneck and the whole kernel is one software pipeline that keeps it
saturated while the PE runs comfortably below it:

- all matmul operands bf16 (fp32 matmuls run at 1/4 PE rate); x is
  pre-transposed on the host (no PE transposes).
- attention is a flat unit list (per q-block: nk k-tiles for row-paired
  heads 0/1, then nk/2 k-tile pairs for self-paired head 2 via a
  swapped [Q2|K2] projection); each unit's score matmuls are emitted
  one unit ahead of its exp, across all pass/q-block boundaries.
- score tiles live in *bf16* PSUM (1 bank per [128,1024] tile), which
  frees enough banks to row-pair the heads-0/1 AV matmuls: the k=0:64
  and k=64:128 halves accumulate concurrently on opposite PE row
  groups into separate banks and are merged during normalize.
- causal band tiles only compute the valid q-range: score and AV
  matmuls shrink by the band offset, and the two narrowest diagonal
  tiles split the exp into two small ones.  Stale PSUM columns are
  never read (AV skips them).
- normalization: DVE approx-reciprocal of the ones-row denominators
  (staged to SBUF), gpsimd partition_broadcast, DVE merge+scale - all
  off the exp critical path, with per-chain PSUM slots so nothing
  blocks accumulation.
- projections for t-block qb+1 and the out-projection of q-block qb-1
  drip between units under explicit deadlines; y^T stages in SBUF and
  leaves in two DMAs per q-block; weight/x DMAs split across both
  hardware DMA queues (sync + scalar).
"""

import sys

sys.path.insert(0, "/opt/trn_rl_repo")

import numpy as np
from contextlib import ExitStack

import concourse.bass as bass
import concourse.bacc as bacc
import concourse.tile as tile
import concourse.mybir as mybir
from concourse.bass_utils import run_bass_kernel_spmd

F32 = mybir.dt.float32
BF16 = mybir.dt.bfloat16
AF = mybir.ActivationFunctionType

B = 2
T = 4096
D = 768
H = 12
DK = 64
NCORES = 8
HL = 3  # heads per core
ND = D // 128  # 6 d-tiles
NKT = T // 128  # 32 k-tiles
NQB = T // 512  # 8 q-blocks

_CACHE = {}


def _emit(tc):
    nc = tc.nc
    xT_d = nc.dram_tensor("xT", [D, T], BF16, kind="ExternalInput").ap()
    wqk_d = nc.dram_tensor("wqkT", [D, 6 * DK], BF16, kind="ExternalInput").ap()
    wv_d = nc.dram_tensor("wvT", [D, HL * DK], BF16, kind="ExternalInput").ap()
    wo01_d = nc.dram_tensor("wo01T", [128, D], BF16, kind="ExternalInput").ap()
    wo2_d = nc.dram_tensor("wo2T", [DK, D], BF16, kind="ExternalInput").ap()
    y_d = nc.dram_tensor("yT", [D, T], F32, kind="ExternalOutput").ap()

    ctx = ExitStack()
    const = ctx.enter_context(tc.tile_pool(name="const", bufs=1))
    persist = ctx.enter_context(tc.tile_pool(name="persist", bufs=1))
    xtpool = ctx.enter_context(tc.tile_pool(name="xt", bufs=2))
    ptpool = ctx.enter_context(tc.tile_pool(name="pt", bufs=6))
    spool = ctx.enter_context(tc.tile_pool(name="sp", bufs=6))
    # PSUM (8 banks): psS tag "ps" 2x[128,1024] = 4 banks (score tiles,
    # also upfront projections + final out-proj); psAV tag "av" 3x 1 bank
    # (av0, av1, av2); tag "x" 1 bank (dripped projections + out-proj).
    psS = ctx.enter_context(tc.tile_pool(name="psS", bufs=2, space="PSUM"))
    psAV = ctx.enter_context(tc.tile_pool(name="psAV", bufs=1, space="PSUM"))

    # ---- weights (split across both HWDGE queues so they load in parallel) ----
    wqk_sb = const.tile([128, ND, 6 * DK], BF16)
    nc.sync.dma_start(out=wqk_sb, in_=wqk_d.rearrange("(j p) e -> p j e", p=128))
    wv_sb = const.tile([128, ND, HL * DK], BF16)
    nc.scalar.dma_start(out=wv_sb, in_=wv_d.rearrange("(j p) e -> p j e", p=128))
    wo01_sb = const.tile([128, D], BF16)
    nc.scalar.dma_start(out=wo01_sb, in_=wo01_d)
    wo2_sb = const.tile([DK, D], BF16)
    nc.scalar.dma_start(out=wo2_sb, in_=wo2_d)

    # warm the exp table set while weights stream in
    warm_in = const.tile([1, 16], F32)
    nc.vector.memset(warm_in, 0.0)
    warm_out = const.tile([1, 16], F32)
    nc.scalar.activation(warm_out, warm_in, AF.Exp, scale=0.125)

    # ---- persistent activations ----
    # KA: [K^T_h0 ; K^T_h1], QB: [Q^T_h0 ; Q^T_h1] on partition halves
    KA = persist.tile([128, T], BF16, name="KA")
    QB = persist.tile([128, T], BF16, name="QB")
    C2 = persist.tile([128, T], BF16, name="C2")  # [K^T_h2 ; Q^T_h2]
    D2 = persist.tile([128, T], BF16, name="D2")  # [Q^T_h2 ; K^T_h2]
    # V natural [t, e] per k-tile with a ones col at e=64 -> softmax sums
    Vall = persist.tile([128, NKT, HL, DK + 1], BF16, name="Vall")
    nc.vector.memset(Vall[:, :, :, DK : DK + 1], 1.0)
    ot01 = persist.tile([128, 512], BF16, name="ot01")  # heads 0/1 out^T
    ot2 = persist.tile([DK, 512], BF16, name="ot2")
    y_acc = persist.tile([128, ND, 512], F32, name="y_acc")  # out-proj staging

    qk_dest = [KA, QB, C2, D2]
    xts = {}

    def emit_xt_dma(tsb):
        tblk = slice(tsb * 512, (tsb + 1) * 512)
        xt_sb = xtpool.tile([128, ND, 512], BF16, name="xt_sb")
        nc.sync.dma_start(
            out=xt_sb, in_=xT_d[:, tblk].rearrange("(j p) t -> p j t", p=128)
        )
        xts[tsb] = xt_sb

    def chunk_q(tsb, et, pool, tag):
        tblk = slice(tsb * 512, (tsb + 1) * 512)

        def thunk():
            xt_sb = xts[tsb]
            ps_q = pool.tile([128, 512], F32, name="ps_q", tag=tag)
            for dj in range(ND):
                nc.tensor.matmul(
                    ps_q,
                    lhsT=wqk_sb[:, dj, et * 128 : (et + 1) * 128],
                    rhs=xt_sb[:, dj, :],
                    start=(dj == 0), stop=(dj == ND - 1),
                )
            nc.vector.tensor_copy(qk_dest[et][:, tblk], ps_q)
            if et == 2:
                # D2 = partition-swapped C2 (self-paired tiling of h2)
                nc.sync.dma_start(out=D2[0:64, tblk], in_=C2[64:128, tblk])
                nc.sync.dma_start(out=D2[64:128, tblk], in_=C2[0:64, tblk])
        return thunk

    def chunk_v(tsb, tt, pool, tag):
        def thunk():
            xt_sb = xts[tsb]
            kt = tsb * 4 + tt
            ps_v = pool.tile([128, HL * DK], F32, name="ps_v", tag=tag)
            for dj in range(ND):
                nc.tensor.matmul(
                    ps_v,
                    lhsT=xt_sb[:, dj, tt * 128 : (tt + 1) * 128],
                    rhs=wv_sb[:, dj, :],
                    start=(dj == 0), stop=(dj == ND - 1),
                )
            nc.vector.tensor_copy(
                Vall[:, kt, :, 0:DK], ps_v.rearrange("p (h e) -> p h e", h=HL)
            )
        return thunk

    # ================= attention =================
    def normalize_one(av, dest):
        sums_sb = spool.tile([1, 512], F32, name="sums_sb")
        nc.vector.tensor_copy(sums_sb, av[DK : DK + 1, :])
        recip = spool.tile([1, 512], F32, name="recip")
        nc.vector.reciprocal_approx_fast(recip, sums_sb)
        recipb = spool.tile([DK, 512], F32, name="recipb")
        nc.gpsimd.partition_broadcast(recipb, recip, channels=DK)
        nc.vector.tensor_mul(dest, av[0:DK, :], recipb)

    def make_outproj(dj, qblk, tag, split_dma):
        def thunk():
            pool = psAV if tag == "x" else psS
            psy = pool.tile([128, 512], F32, name="psy", tag=tag)
            nc.tensor.matmul(
                psy, lhsT=wo01_sb[:, dj * 128 : (dj + 1) * 128], rhs=ot01,
                start=True, stop=False, skip_group_check=True,
            )
            nc.tensor.matmul(
                psy, lhsT=wo2_sb[:, dj * 128 : (dj + 1) * 128], rhs=ot2,
                start=False, stop=True, skip_group_check=True,
            )
            nc.vector.tensor_copy(y_acc[:, dj, :], psy)
            if split_dma:  # last q-block: overlap DMA with the serial tail
                nc.sync.dma_start(
                    out=y_d[dj * 128 : (dj + 1) * 128, qblk],
                    in_=y_acc[:, dj, :],
                )
            elif dj in (2, ND - 1):
                half = slice(0, 3) if dj == 2 else slice(3, ND)
                rows = slice(half.start * 128, half.stop * 128)
                nc.sync.dma_start(
                    out=y_d[rows, qblk].rearrange("(j p) q -> p j q", p=128),
                    in_=y_acc[:, half, :],
                )
        return thunk

    # Flat unit list; each unit emits its scores one unit ahead of its exp.
    units = []
    for qb in range(NQB):
        nk = 4 * (qb + 1)
        qblk0 = qb * 512
        state = {}

        def mk_scores01(kt, qb=qb, qblk0=qblk0):
            band = kt - 4 * qb
            o = 128 * band if band > 0 else 0  # valid q-range is [o, 512)

            def emit():
                pss = psS.tile([128, 1024], F32, name="pss", tag="ps")
                kblk = slice(kt * 128, (kt + 1) * 128)
                qs = slice(qblk0 + o, qblk0 + 512)
                nc.tensor.matmul(
                    pss[:, o:512], lhsT=KA[0:64, kblk], rhs=QB[0:64, qs],
                    start=True, stop=True,
                )
                nc.tensor.matmul(
                    pss[:, 512 + o : 1024], lhsT=KA[64:128, kblk],
                    rhs=QB[64:128, qs], start=True, stop=True,
                )
                return pss
            return emit

        def mk_proc01(kt, qb=qb, nk=nk, state=state):
            band = kt - 4 * qb
            o = 128 * band if band > 0 else 0

            def proc(pss):
                if kt == 0:
                    state["av"] = [
                        psAV.tile([DK + 1, 512], F32, name=f"av{h}",
                                  tag="av", bufs=3)
                        for h in (0, 1)
                    ]
                pt = ptpool.tile([128, 1024], BF16, name="pt")
                if band >= 2:  # two narrow exps skip the masked-out range
                    nc.scalar.activation(
                        pt[:, o:512], pss[:, o:512], AF.Exp, scale=0.125
                    )
                    nc.scalar.activation(
                        pt[:, 512 + o : 1024], pss[:, 512 + o : 1024],
                        AF.Exp, scale=0.125,
                    )
                else:
                    nc.scalar.activation(pt, pss, AF.Exp, scale=0.125)
                if band >= 0:
                    # causal mask: within the trimmed region, keep col >= k
                    # (the trim already starts at the band offset)
                    for half in (0, 1):
                        pv = pt[:, half * 512 + o : (half + 1) * 512]
                        nc.gpsimd.affine_select(
                            out=pv, in_=pv,
                            compare_op=mybir.AluOpType.is_ge, fill=0.0,
                            base=0, pattern=[[1, 512 - o]],
                            channel_multiplier=-1,
                        )
                for h in (0, 1):
                    nc.tensor.matmul(
                        state["av"][h][:, o:512],
                        lhsT=Vall[:, kt, h, :],
                        rhs=pt[:, h * 512 + o : (h + 1) * 512],
                        start=(kt == 0), stop=(kt == nk - 1),
                    )
            return proc

        def mk_scores2(kp, qb=qb, qblk0=qblk0):
            b0 = 2 * kp - 4 * qb
            o0 = 128 * b0 if b0 > 0 else 0
            o1 = 128 * (b0 + 1) if b0 + 1 > 0 else 0

            def emit():
                pss = psS.tile([128, 1024], F32, name="pss2", tag="ps")
                kb0 = slice((2 * kp) * 128, (2 * kp + 1) * 128)
                kb1 = slice((2 * kp + 1) * 128, (2 * kp + 2) * 128)
                nc.tensor.matmul(
                    pss[:, o0:512], lhsT=C2[0:64, kb0],
                    rhs=D2[0:64, qblk0 + o0 : qblk0 + 512],
                    start=True, stop=True,
                )
                nc.tensor.matmul(
                    pss[:, 512 + o1 : 1024], lhsT=D2[64:128, kb1],
                    rhs=C2[64:128, qblk0 + o1 : qblk0 + 512],
                    start=True, stop=True,
                )
                return pss
            return emit

        def mk_proc2(kp, qb=qb, nk=nk, state=state):
            nkp = nk // 2
            b0 = 2 * kp - 4 * qb
            o0 = 128 * b0 if b0 > 0 else 0
            o1 = 128 * (b0 + 1) if b0 + 1 > 0 else 0

            def proc(pss):
                if kp == 0:
                    state["av2"] = psAV.tile(
                        [DK + 1, 512], F32, name="av2", tag="av", bufs=3
                    )
                pt2 = ptpool.tile([128, 1024], BF16, name="pt2")
                if b0 >= 2:
                    nc.scalar.activation(
                        pt2[:, o0:512], pss[:, o0:512], AF.Exp, scale=0.125
                    )
                    nc.scalar.activation(
                        pt2[:, 512 + o1 : 1024], pss[:, 512 + o1 : 1024],
                        AF.Exp, scale=0.125,
                    )
                else:
                    nc.scalar.activation(pt2, pss, AF.Exp, scale=0.125)
                if b0 >= 0:
                    for oo, base_col in ((o0, 0), (o1, 512)):
                        pv = pt2[:, base_col + oo : base_col + 512]
                        nc.gpsimd.affine_select(
                            out=pv, in_=pv,
                            compare_op=mybir.AluOpType.is_ge, fill=0.0,
                            base=0, pattern=[[1, 512 - oo]],
                            channel_multiplier=-1,
                        )
                nc.tensor.matmul(
                    state["av2"][:, o0:512], lhsT=Vall[:, 2 * kp, 2, :],
                    rhs=pt2[:, o0:512], start=(kp == 0), stop=False,
                )
                nc.tensor.matmul(
                    state["av2"][:, o1:512], lhsT=Vall[:, 2 * kp + 1, 2, :],
                    rhs=pt2[:, 512 + o1 : 1024], start=False,
                    stop=(kp == nkp - 1),
                )
            return proc

        for kt in range(nk):
            units.append(
                dict(qb=qb, first=(kt == 0), last=False,
                     hp0_last=(kt == nk - 1),
                     qblk=slice(qblk0, qblk0 + 512), state=state,
                     scores=mk_scores01(kt), proc=mk_proc01(kt))
            )
        for kp in range(nk // 2):
            units.append(
                dict(qb=qb, first=False, last=(kp == nk // 2 - 1),
                     hp0_last=False,
                     qblk=slice(qblk0, qblk0 + 512), state=state,
                     scores=mk_scores2(kp), proc=mk_proc2(kp))
            )

    u_start = {}
    for i, u in enumerate(units):
        u_start.setdefault(u["qb"], i)

    # ---- prologue: x/weights stream in, first projections go through the
    # idle psS pool; C2/D2/V chunks get deadlines and drip into q-block 0.
    emit_xt_dma(0)
    if NQB > 1:
        emit_xt_dma(1)
    chunk_q(0, 0, psS, "ps")()
    chunk_q(0, 1, psS, "ps")()

    pending = []  # (due_unit, thunk), kept sorted by insertion order

    def push(due, thunk):
        pending.append((due, thunk))

    push(0, chunk_v(0, 0, psAV, "x"))
    push(1, chunk_v(0, 1, psAV, "x"))
    push(1, chunk_q(0, 2, psAV, "x"))  # C2+D2 needed by unit 4's scores
    push(2, chunk_v(0, 2, psAV, "x"))
    push(3, chunk_v(0, 3, psAV, "x"))

    pss_next = units[0]["scores"]()
    for u_idx, u in enumerate(units):
        qb = u["qb"]
        if u["first"]:
            if qb + 2 < NQB:
                emit_xt_dma(qb + 2)
            if qb + 1 < NQB:
                # projections for t-block qb+1: Q/K parts are needed by the
                # next q-block's first scores (emitted one unit before the
                # boundary); V parts by the AV of the new k-tiles in qb+1.
                nqb_start = u_start[qb + 1]
                for et in range(3):
                    push(nqb_start - 2 - (2 - et), chunk_q(qb + 1, et, psAV, "x"))
                for tt in range(4):
                    push(nqb_start + 4 * qb + 4 + tt, chunk_v(qb + 1, tt, psAV, "x"))
        pss_cur = pss_next
        if u_idx + 1 < len(units):
            pss_next = units[u_idx + 1]["scores"]()
        # drip: pop everything due; else stay one ahead
        popped = False
        while pending and pending[0][0] <= u_idx:
            pending.pop(0)[1]()
            popped = True
        if pending and not popped:
            pending.pop(0)[1]()
        u["proc"](pss_cur)
        if u["hp0_last"]:
            # heads 0/1 finish here; normalize them while head 2 runs
            st = u["state"]
            normalize_one(st["av"][0], ot01[0:DK, :])
            ot1s = spool.tile([DK, 512], BF16, name="ot1s")
            normalize_one(st["av"][1], ot1s)
            nc.sync.dma_start(out=ot01[DK:128, :], in_=ot1s)
        if u["last"]:
            st = u["state"]
            normalize_one(st["av2"], ot2)
            final = qb == NQB - 1
            tag = "ps" if final else "x"
            for dj in range(ND):
                # out-proj of this q-block: ot01/ot2 are rewritten at the
                # *next* q-block's hp0 end, so it must land before that
                due = (len(units) if final
                       else u_start[qb + 1] + 4 * (qb + 2) - 1 - (ND - 1 - dj))
                push(due, make_outproj(dj, u["qblk"], tag, final))

    for _, thunk in pending:
        thunk()
    ctx.close()


def build():
    if "nc" in _CACHE:
        return _CACHE["nc"]
    nc = bacc.Bacc(
        "TRN2", target_bir_lowering=False, debug=False, num_devices=NCORES
    )
    with tile.TileContext(nc) as tc:
        _emit(tc)
    nc.compile()
    _CACHE["nc"] = nc
    return nc


def make_in_maps(x, w_qkv, w_out):
    import ml_dtypes

    bf16 = ml_dtypes.bfloat16
    x = np.asarray(x, dtype=np.float32)
    w_qkv = np.asarray(w_qkv, dtype=np.float32)
    w_out = np.asarray(w_out, dtype=np.float32)
    wq = w_qkv[0:D]        # [768, 768], rows = q features
    wk = w_qkv[D : 2 * D]
    wv = w_qkv[2 * D :]
    in_maps = []
    for c in range(NCORES):
        b, g = divmod(c, 4)
        hs = [3 * g + j for j in range(HL)]  # global head ids
        h0, h1, h2 = hs
        cols = []
        # e-tiles: [K0|K1] -> KA, [Q0|Q1] -> QB, [K2|Q2] -> C2, [Q2|K2] -> D2
        for pair in ((wk, h0), (wk, h1), (wq, h0), (wq, h1),
                     (wk, h2), (wq, h2)):
            w, h = pair
            cols.append(w[h * DK : (h + 1) * DK].T)  # [768, 64]
        wqkT = np.concatenate(cols, axis=1).astype(bf16)  # [768, 384]
        wvT = np.concatenate(
            [wv[h * DK : (h + 1) * DK].T for h in hs], axis=1
        ).astype(bf16)  # [768, 192]
        wo01T = np.concatenate(
            [w_out[:, h * DK : (h + 1) * DK].T for h in (h0, h1)], axis=0
        ).astype(bf16)  # [128, 768]
        wo2T = w_out[:, h2 * DK : (h2 + 1) * DK].T.astype(bf16)  # [64, 768]
        xT = np.ascontiguousarray(x[b].T).astype(bf16)  # [768, 4096]
        in_maps.append(
            {"xT": xT, "wqkT": wqkT, "wvT": wvT, "wo01T": wo01T, "wo2T": wo2T}
        )
    return in_maps


def run(inputs, trace=False):
    """Run on hardware; returns (y [B,T,D] fp32, BassKernelResults)."""
    nc = build()
    in_maps = make_in_maps(inputs["x"], inputs["w_qkv"], inputs["w_out"])
    br = run_bass_kernel_spmd(nc, in_maps, list(range(NCORES)), trace=trace)
    y = np.zeros((B, T, D), dtype=np.float32)
    for c in range(NCORES):
        b = c // 4
        y[b] += np.asarray(br.results[c]["yT"]).T
    return y, br


def kernel(x, w_qkv, w_out):
    y, _ = run({"x": x, "w_qkv": w_qkv, "w_out": w_out})
    return y


# revision 38
# speedup vs baseline: 1.0547x; 1.0547x over previous
"""Multi-head causal self-attention (B=2, T=4096, D=768, H=12) on 8 trn2 cores.

Sharding: core c -> batch b = c//4, heads 3*(c%4) .. 3*(c%4)+2.
qkv_proj column-parallel (each core computes Q/K/V only for its heads),
out_proj row-parallel (each core emits a partial y^T; host sums the 4
partials per batch).

v6: the ScalarE exp stream (~1us per [128,1024] score tile) is the
bottleneck and the whole kernel is one software pipeline that keeps it
saturated while the PE runs comfortably below it:

- all matmul operands bf16 (fp32 matmuls run at 1/4 PE rate); x is
  pre-transposed on the host (no PE transposes).
- attention is a flat unit list (per q-block: nk k-tiles for row-paired
  heads 0/1, then nk/2 k-tile pairs for self-paired head 2 via a
  swapped [Q2|K2] projection); each unit's score matmuls are emitted
  one unit ahead of its exp, across all pass/q-block boundaries.
- score tiles live in *bf16* PSUM (1 bank per [128,1024] tile), which
  frees enough banks to row-pair the heads-0/1 AV matmuls: the k=0:64
  and k=64:128 halves accumulate concurrently on opposite PE row
  groups into separate banks and are merged during normalize.
- causal band tiles only compute the valid q-range: score and AV
  matmuls shrink by the band offset, and the two narrowest diagonal
  tiles split the exp into two small ones.  Stale PSUM columns are
  never read (AV skips them).
- normalization: DVE approx-reciprocal of the ones-row denominators
  (staged to SBUF), gpsimd partition_broadcast, DVE merge+scale - all
  off the exp critical path, with per-chain PSUM slots so nothing
  blocks accumulation.
- projections for t-block qb+1 and the out-projection of q-block qb-1
  drip between units under explicit deadlines; y^T stages in SBUF and
  leaves in two DMAs per q-block; weight/x DMAs split across both
  hardware DMA queues (sync + scalar).
"""

import sys

sys.path.insert(0, "/opt/trn_rl_repo")

import numpy as np
from contextlib import ExitStack

import concourse.bass as bass
import concourse.bacc as bacc
import concourse.tile as tile
import concourse.mybir as mybir
from concourse.bass_utils import run_bass_kernel_spmd

F32 = mybir.dt.float32
BF16 = mybir.dt.bfloat16
AF = mybir.ActivationFunctionType

B = 2
T = 4096
D = 768
H = 12
DK = 64
NCORES = 8
HL = 3  # heads per core
ND = D // 128  # 6 d-tiles
NKT = T // 128  # 32 k-tiles
NQB = T // 512  # 8 q-blocks

_CACHE = {}


def _emit(tc):
    nc = tc.nc
    xT_d = nc.dram_tensor("xT", [D, T], BF16, kind="ExternalInput").ap()
    wqk_d = nc.dram_tensor("wqkT", [D, 6 * DK], BF16, kind="ExternalInput").ap()
    wv_d = nc.dram_tensor("wvT", [D, HL * DK], BF16, kind="ExternalInput").ap()
    wo01_d = nc.dram_tensor("wo01T", [128, D], BF16, kind="ExternalInput").ap()
    wo2_d = nc.dram_tensor("wo2T", [DK, D], BF16, kind="ExternalInput").ap()
    y_d = nc.dram_tensor("yT", [D, T], F32, kind="ExternalOutput").ap()

    ctx = ExitStack()
    const = ctx.enter_context(tc.tile_pool(name="const", bufs=1))
    persist = ctx.enter_context(tc.tile_pool(name="persist", bufs=1))
    xtpool = ctx.enter_context(tc.tile_pool(name="xt", bufs=2))
    ptpool = ctx.enter_context(tc.tile_pool(name="pt", bufs=6))
    spool = ctx.enter_context(tc.tile_pool(name="sp", bufs=6))
    # PSUM (8 banks): psS tag "ps" 2x[128,1024] = 4 banks (score tiles,
    # also upfront projections + final out-proj); psAV tag "av" 3x 1 bank
    # (av0, av1, av2); tag "x" 1 bank (dripped projections + out-proj).
    psS = ctx.enter_context(tc.tile_pool(name="psS", bufs=2, space="PSUM"))
    psAV = ctx.enter_context(tc.tile_pool(name="psAV", bufs=1, space="PSUM"))

    xts = {}

    def emit_xt_dma(tsb, queue=None):
        tblk = slice(tsb * 512, (tsb + 1) * 512)
        xt_sb = xtpool.tile([128, ND, 512], BF16, name="xt_sb")
        (queue or nc.sync).dma_start(
            out=xt_sb, in_=xT_d[:, tblk].rearrange("(j p) t -> p j t", p=128)
        )
        xts[tsb] = xt_sb

    # ---- weights (split across both HWDGE queues so they load in parallel;
    # x^T block 0 rides the scalar queue so the first projection can start
    # as soon as wqk lands) ----
    wqk_sb = const.tile([128, ND, 6 * DK], BF16)
    nc.sync.dma_start(out=wqk_sb, in_=wqk_d.rearrange("(j p) e -> p j e", p=128))
    emit_xt_dma(0, nc.scalar)
    wv_sb = const.tile([128, ND, HL * DK], BF16)
    nc.scalar.dma_start(out=wv_sb, in_=wv_d.rearrange("(j p) e -> p j e", p=128))
    wo01_sb = const.tile([128, D], BF16)
    nc.scalar.dma_start(out=wo01_sb, in_=wo01_d)
    wo2_sb = const.tile([DK, D], BF16)
    nc.scalar.dma_start(out=wo2_sb, in_=wo2_d)

    # warm the exp table set while weights stream in
    warm_in = const.tile([1, 16], F32)
    nc.vector.memset(warm_in, 0.0)
    warm_out = const.tile([1, 16], F32)
    nc.scalar.activation(warm_out, warm_in, AF.Exp, scale=0.125)

    # master causal mask: M[k, 512+c] = (c >= k).  Every band tile, after
    # its trim, masks with the same slice M[:, 512:512+w].
    M = const.tile([128, 1024], BF16, name="M")
    nc.gpsimd.memset(M, 1.0)
    nc.gpsimd.affine_select(
        out=M, in_=M, compare_op=mybir.AluOpType.is_ge, fill=0.0,
        base=-512, pattern=[[1, 1024]], channel_multiplier=-1,
    )

    # warm the PE clock (HAM un-throttles after ~3.4us of activity) while
    # the x/weight DMAs are in flight
    warm_mm = const.tile([128, 128], BF16, name="warm_mm")
    nc.vector.memset(warm_mm, 0.0)
    for _ in range(2):
        warm_ps = psS.tile([128, 512], F32, name="warm_ps", tag="ps")
        for _i in range(16):
            nc.tensor.matmul(
                warm_ps[:, 0:128], lhsT=warm_mm, rhs=warm_mm,
                start=True, stop=True, skip_group_check=True,
            )

    # ---- persistent activations ----
    # KA: [K^T_h0 ; K^T_h1], QB: [Q^T_h0 ; Q^T_h1] on partition halves
    KA = persist.tile([128, T], BF16, name="KA")
    QB = persist.tile([128, T], BF16, name="QB")
    C2 = persist.tile([128, T], BF16, name="C2")  # [K^T_h2 ; Q^T_h2]
    D2 = persist.tile([128, T], BF16, name="D2")  # [Q^T_h2 ; K^T_h2]
    # V natural [t, e] per k-tile with a ones col at e=64 -> softmax sums
    Vall = persist.tile([128, NKT, HL, DK + 1], BF16, name="Vall")
    nc.vector.memset(Vall[:, :, :, DK : DK + 1], 1.0)
    ot01 = persist.tile([128, 512], BF16, name="ot01")  # heads 0/1 out^T
    ot2 = persist.tile([DK, 512], BF16, name="ot2")
    y_acc = persist.tile([128, ND, 512], F32, name="y_acc")  # out-proj staging

    qk_dest = [KA, QB, C2, D2]

    def chunk_q(tsb, et, pool, tag):
        tblk = slice(tsb * 512, (tsb + 1) * 512)

        def thunk():
            xt_sb = xts[tsb]
            ps_q = pool.tile([128, 512], F32, name="ps_q", tag=tag)
            for dj in range(ND):
                nc.tensor.matmul(
                    ps_q,
                    lhsT=wqk_sb[:, dj, et * 128 : (et + 1) * 128],
                    rhs=xt_sb[:, dj, :],
                    start=(dj == 0), stop=(dj == ND - 1),
                )
            nc.vector.tensor_copy(qk_dest[et][:, tblk], ps_q)
            if et == 2:
                # D2 = partition-swapped C2 (self-paired tiling of h2)
                nc.sync.dma_start(out=D2[0:64, tblk], in_=C2[64:128, tblk])
                nc.sync.dma_start(out=D2[64:128, tblk], in_=C2[0:64, tblk])
        return thunk

    def chunk_v(tsb, tt, pool, tag):
        def thunk():
            xt_sb = xts[tsb]
            kt = tsb * 4 + tt
            ps_v = pool.tile([128, HL * DK], F32, name="ps_v", tag=tag)
            for dj in range(ND):
                nc.tensor.matmul(
                    ps_v,
                    lhsT=xt_sb[:, dj, tt * 128 : (tt + 1) * 128],
                    rhs=wv_sb[:, dj, :],
                    start=(dj == 0), stop=(dj == ND - 1),
                )
            nc.vector.tensor_copy(
                Vall[:, kt, :, 0:DK], ps_v.rearrange("p (h e) -> p h e", h=HL)
            )
        return thunk

    # ================= attention =================
    def normalize_one(av, dest):
        sums_sb = spool.tile([1, 512], F32, name="sums_sb")
        nc.vector.tensor_copy(sums_sb, av[DK : DK + 1, :])
        recip = spool.tile([1, 512], F32, name="recip")
        nc.vector.reciprocal_approx_fast(recip, sums_sb)
        recipb = spool.tile([DK, 512], F32, name="recipb")
        nc.gpsimd.partition_broadcast(recipb, recip, channels=DK)
        nc.vector.tensor_mul(dest, av[0:DK, :], recipb)

    def make_outproj(dj, qblk, tag, split_dma):
        def thunk():
            pool = psAV if tag == "x" else psS
            psy = pool.tile([128, 512], F32, name="psy", tag=tag)
            nc.tensor.matmul(
                psy, lhsT=wo01_sb[:, dj * 128 : (dj + 1) * 128], rhs=ot01,
                start=True, stop=False, skip_group_check=True,
            )
            nc.tensor.matmul(
                psy, lhsT=wo2_sb[:, dj * 128 : (dj + 1) * 128], rhs=ot2,
                start=False, stop=True, skip_group_check=True,
            )
            nc.vector.tensor_copy(y_acc[:, dj, :], psy)
            if split_dma:  # last q-block: overlap DMA with the serial tail
                nc.sync.dma_start(
                    out=y_d[dj * 128 : (dj + 1) * 128, qblk],
                    in_=y_acc[:, dj, :],
                )
            elif dj in (2, ND - 1):
                half = slice(0, 3) if dj == 2 else slice(3, ND)
                rows = slice(half.start * 128, half.stop * 128)
                nc.sync.dma_start(
                    out=y_d[rows, qblk].rearrange("(j p) q -> p j q", p=128),
                    in_=y_acc[:, half, :],
                )
        return thunk

    # Flat unit list; each unit emits its scores one unit ahead of its exp.
    units = []
    for qb in range(NQB):
        nk = 4 * (qb + 1)
        qblk0 = qb * 512
        state = {}

        def mk_scores01(kt, qb=qb, qblk0=qblk0):
            band = kt - 4 * qb
            o = 128 * band if band > 0 else 0  # valid q-range is [o, 512)

            def emit():
                pss = psS.tile([128, 1024], F32, name="pss", tag="ps")
                kblk = slice(kt * 128, (kt + 1) * 128)
                qs = slice(qblk0 + o, qblk0 + 512)
                nc.tensor.matmul(
                    pss[:, o:512], lhsT=KA[0:64, kblk], rhs=QB[0:64, qs],
                    start=True, stop=True,
                )
                nc.tensor.matmul(
                    pss[:, 512 + o : 1024], lhsT=KA[64:128, kblk],
                    rhs=QB[64:128, qs], start=True, stop=True,
                )
                return pss
            return emit

        def mk_proc01(kt, qb=qb, nk=nk, state=state):
            band = kt - 4 * qb
            o = 128 * band if band > 0 else 0

            def proc(pss):
                if kt == 0:
                    state["av"] = [
                        psAV.tile([DK + 1, 512], F32, name=f"av{h}",
                                  tag="av", bufs=3)
                        for h in (0, 1)
                    ]
                pt = ptpool.tile([128, 1024], BF16, name="pt")
                if band >= 2:  # two narrow exps skip the masked-out range
                    nc.scalar.activation(
                        pt[:, o:512], pss[:, o:512], AF.Exp, scale=0.125
                    )
                    nc.scalar.activation(
                        pt[:, 512 + o : 1024], pss[:, 512 + o : 1024],
                        AF.Exp, scale=0.125,
                    )
                else:
                    nc.scalar.activation(pt, pss, AF.Exp, scale=0.125)
                if band >= 0:
                    # causal mask: within the trimmed region, keep col >= k
                    # (the trim already starts at the band offset)
                    ms = M[:, 512 : 1024 - o]
                    for half in (0, 1):
                        pv = pt[:, half * 512 + o : (half + 1) * 512]
                        nc.vector.tensor_mul(pv, pv, ms)
                for h in (0, 1):
                    nc.tensor.matmul(
                        state["av"][h][:, o:512],
                        lhsT=Vall[:, kt, h, :],
                        rhs=pt[:, h * 512 + o : (h + 1) * 512],
                        start=(kt == 0), stop=(kt == nk - 1),
                    )
            return proc

        def mk_scores2(kp, qb=qb, qblk0=qblk0):
            b0 = 2 * kp - 4 * qb
            o0 = 128 * b0 if b0 > 0 else 0
            o1 = 128 * (b0 + 1) if b0 + 1 > 0 else 0

            def emit():
                pss = psS.tile([128, 1024], F32, name="pss2", tag="ps")
                kb0 = slice((2 * kp) * 128, (2 * kp + 1) * 128)
                kb1 = slice((2 * kp + 1) * 128, (2 * kp + 2) * 128)
                nc.tensor.matmul(
                    pss[:, o0:512], lhsT=C2[0:64, kb0],
                    rhs=D2[0:64, qblk0 + o0 : qblk0 + 512],
                    start=True, stop=True,
                )
                nc.tensor.matmul(
                    pss[:, 512 + o1 : 1024], lhsT=D2[64:128, kb1],
                    rhs=C2[64:128, qblk0 + o1 : qblk0 + 512],
                    start=True, stop=True,
                )
                return pss
            return emit

        def mk_proc2(kp, qb=qb, nk=nk, state=state):
            nkp = nk // 2
            b0 = 2 * kp - 4 * qb
            o0 = 128 * b0 if b0 > 0 else 0
            o1 = 128 * (b0 + 1) if b0 + 1 > 0 else 0

            def proc(pss):
                if kp == 0:
                    state["av2"] = psAV.tile(
                        [DK + 1, 512], F32, name="av2", tag="av", bufs=3
                    )
                pt2 = ptpool.tile([128, 1024], BF16, name="pt2")
                if b0 >= 2:
                    nc.scalar.activation(
                        pt2[:, o0:512], pss[:, o0:512], AF.Exp, scale=0.125
                    )
                    nc.scalar.activation(
                        pt2[:, 512 + o1 : 1024], pss[:, 512 + o1 : 1024],
                        AF.Exp, scale=0.125,
                    )
                else:
                    nc.scalar.activation(pt2, pss, AF.Exp, scale=0.125)
                if b0 >= 0:
                    for oo, base_col in ((o0, 0), (o1, 512)):
                        pv = pt2[:, base_col + oo : base_col + 512]
                        nc.vector.tensor_mul(pv, pv, M[:, 512 : 1024 - oo])
                nc.tensor.matmul(
                    state["av2"][:, o0:512], lhsT=Vall[:, 2 * kp, 2, :],
                    rhs=pt2[:, o0:512], start=(kp == 0), stop=False,
                )
                nc.tensor.matmul(
                    state["av2"][:, o1:512], lhsT=Vall[:, 2 * kp + 1, 2, :],
                    rhs=pt2[:, 512 + o1 : 1024], start=False,
                    stop=(kp == nkp - 1),
                )
            return proc

        for kt in range(nk):
            units.append(
                dict(qb=qb, first=(kt == 0), last=False,
                     hp0_last=(kt == nk - 1),
                     qblk=slice(qblk0, qblk0 + 512), state=state,
                     scores=mk_scores01(kt), proc=mk_proc01(kt))
            )
        for kp in range(nk // 2):
            units.append(
                dict(qb=qb, first=False, last=(kp == nk // 2 - 1),
                     hp0_last=False,
                     qblk=slice(qblk0, qblk0 + 512), state=state,
                     scores=mk_scores2(kp), proc=mk_proc2(kp))
            )

    u_start = {}
    for i, u in enumerate(units):
        u_start.setdefault(u["qb"], i)

    # ---- prologue: x/weights stream in, first projections go through the
    # idle psS pool; C2/D2/V chunks get deadlines and drip into q-block 0.
    if NQB > 1:
        emit_xt_dma(1)
    chunk_q(0, 0, psS, "ps")()
    chunk_q(0, 1, psS, "ps")()

    pending = []  # (due_unit, thunk), kept sorted by insertion order

    def push(due, thunk):
        pending.append((due, thunk))

    push(0, chunk_v(0, 0, psAV, "x"))
    push(1, chunk_v(0, 1, psAV, "x"))
    push(1, chunk_q(0, 2, psAV, "x"))  # C2+D2 needed by unit 4's scores
    push(2, chunk_v(0, 2, psAV, "x"))
    push(3, chunk_v(0, 3, psAV, "x"))

    pss_next = units[0]["scores"]()
    for u_idx, u in enumerate(units):
        qb = u["qb"]
        if u["first"]:
            if qb + 2 < NQB:
                emit_xt_dma(qb + 2)
            if qb + 1 < NQB:
                # projections for t-block qb+1: Q/K parts are needed by the
                # next q-block's first scores (emitted one unit before the
                # boundary); V parts by the AV of the new k-tiles in qb+1.
                nqb_start = u_start[qb + 1]
                for et in range(3):
                    push(nqb_start - 2 - (2 - et), chunk_q(qb + 1, et, psAV, "x"))
                for tt in range(4):
                    push(nqb_start + 4 * qb + 4 + tt, chunk_v(qb + 1, tt, psAV, "x"))
        pss_cur = pss_next
        if u_idx + 1 < len(units):
            pss_next = units[u_idx + 1]["scores"]()
        # drip: pop everything due; else stay one ahead
        popped = False
        while pending and pending[0][0] <= u_idx:
            pending.pop(0)[1]()
            popped = True
        if pending and not popped:
            pending.pop(0)[1]()
        u["proc"](pss_cur)
        if u["hp0_last"]:
            # heads 0/1 finish here; normalize them while head 2 runs
            st = u["state"]
            normalize_one(st["av"][0], ot01[0:DK, :])
            ot1s = spool.tile([DK, 512], BF16, name="ot1s")
            normalize_one(st["av"][1], ot1s)
            nc.sync.dma_start(out=ot01[DK:128, :], in_=ot1s)
        if u["last"]:
            st = u["state"]
            normalize_one(st["av2"], ot2)
            final = qb == NQB - 1
            tag = "ps" if final else "x"
            for dj in range(ND):
                # out-proj of this q-block: ot01/ot2 are rewritten at the
                # *next* q-block's hp0 end, so it must land before that
                due = (len(units) if final
                       else u_start[qb + 1] + 4 * (qb + 2) - 1 - (ND - 1 - dj))
                push(due, make_outproj(dj, u["qblk"], tag, final))

    for _, thunk in pending:
        thunk()
    ctx.close()


def build():
    if "nc" in _CACHE:
        return _CACHE["nc"]
    nc = bacc.Bacc(
        "TRN2", target_bir_lowering=False, debug=False, num_devices=NCORES
    )
    with tile.TileContext(nc) as tc:
        _emit(tc)
    nc.compile()
    _CACHE["nc"] = nc
    return nc


def make_in_maps(x, w_qkv, w_out):
    import ml_dtypes

    bf16 = ml_dtypes.bfloat16
    x = np.asarray(x, dtype=np.float32)
    w_qkv = np.asarray(w_qkv, dtype=np.float32)
    w_out = np.asarray(w_out, dtype=np.float32)
    wq = w_qkv[0:D]        # [768, 768], rows = q features
    wk = w_qkv[D : 2 * D]
    wv = w_qkv[2 * D :]
    in_maps = []
    for c in range(NCORES):
        b, g = divmod(c, 4)
        hs = [3 * g + j for j in range(HL)]  # global head ids
        h0, h1, h2 = hs
        cols = []
        # e-tiles: [K0|K1] -> KA, [Q0|Q1] -> QB, [K2|Q2] -> C2, [Q2|K2] -> D2
        for pair in ((wk, h0), (wk, h1), (wq, h0), (wq, h1),
                     (wk, h2), (wq, h2)):
            w, h = pair
            cols.append(w[h * DK : (h + 1) * DK].T)  # [768, 64]
        wqkT = np.concatenate(cols, axis=1).astype(bf16)  # [768, 384]
        wvT = np.concatenate(
            [wv[h * DK : (h + 1) * DK].T for h in hs], axis=1
        ).astype(bf16)  # [768, 192]
        wo01T = np.concatenate(
            [w_out[:, h * DK : (h + 1) * DK].T for h in (h0, h1)], axis=0
        ).astype(bf16)  # [128, 768]
        wo2T = w_out[:, h2 * DK : (h2 + 1) * DK].T.astype(bf16)  # [64, 768]
        xT = np.ascontiguousarray(x[b].T).astype(bf16)  # [768, 4096]
        in_maps.append(
            {"xT": xT, "wqkT": wqkT, "wvT": wvT, "wo01T": wo01T, "wo2T": wo2T}
        )
    return in_maps


def run(inputs, trace=False):
    """Run on hardware; returns (y [B,T,D] fp32, BassKernelResults)."""
    nc = build()
    in_maps = make_in_maps(inputs["x"], inputs["w_qkv"], inputs["w_out"])
    br = run_bass_kernel_spmd(nc, in_maps, list(range(NCORES)), trace=trace)
    y = np.zeros((B, T, D), dtype=np.float32)
    for c in range(NCORES):
        b = c // 4
        y[b] += np.asarray(br.results[c]["yT"]).T
    return y, br


def kernel(x, w_qkv, w_out):
    y, _ = run({"x": x, "w_qkv": w_qkv, "w_out": w_out})
    return y
